# revision 1
# baseline (speedup 1.0000x reference)
"""MixHopNet Trainium2 kernel: 8-core SPMD, node-sharded.

Math (matches the jax reference):
  dinv = rsqrt(deg) with self loops; Ahat = D (A+I) D  (D = diag(dinv))
  h  = relu([x W0, (Ahat x) W1, (Ahat^2 x) W2] + b1)
  y  = [h V0, Ahat (h V1)] + b2
  out = log_softmax(y, axis=1)

Implementation notes:
  - norm is separable (dinv[row]*dinv[col]) -> propagation is a 0/1 adjacency
    sum over pre-scaled features; self loops are fused scale-adds at PSUM evac.
  - gather: gpsimd.dma_gather (int16 idxs -> 32k-row windows), fp16 rows 256B.
  - scatter: one-hot matmul on TensorE. S[128 edges, 512 dests] generated on
    VectorE via is_equal(dlocal, iota512); PSUM out is feature-major
    [128 feat, 512 dest] accumulated over all chunks of a 512-dest group.
  - one program for all 8 cores: per-(destgroup, window) chunk counts are
    equalized to the cross-core max (pad slots gather row 0 and have
    dlocal=-1 so they contribute nothing).
  - all-gathers of the propagated features between steps (ncfw collectives).
"""

from contextlib import ExitStack

import numpy as np
import ml_dtypes

import concourse.bass as bass
import concourse.mybir as mybir
import concourse.tile as tile
from concourse import bacc, bass_utils

FP16 = mybir.dt.float16
F32 = mybir.dt.float32
I16 = mybir.dt.int16

N_CORES = 8
F_IN, HID, NCLS = 128, 512, 40
WIN = 32768
NBLK = 512


def make_dims(n):
    sl = -(-n // (N_CORES * 512)) * 512       # per-core slice, multiple of 512
    npad = sl * N_CORES
    return dict(
        N=n, SL=sl, NP=npad,
        NT=sl // 128,            # 128-row dest tiles per core
        NS4=sl // 512,           # 512-dest groups per core
        NW=-(-npad // WIN),      # source windows
        NB=sl // NBLK,           # GEMM n-blocks
    )


# ================================================================ host prep

def _wrap16(seg):
    return np.ascontiguousarray(seg.reshape(-1, 16).T)


def host_prep(x, edge_index, w1, b1, w2, b2, dims):
    N, SL, NP, NS4, NW = dims["N"], dims["SL"], dims["NP"], dims["NS4"], dims["NW"]
    NT = dims["NT"]
    x = np.asarray(x, np.float32)
    ei = np.asarray(edge_index)
    row, col = ei[0].astype(np.int64), ei[1].astype(np.int64)

    deg = np.bincount(col, minlength=N).astype(np.float32) + 1.0
    dinv = (1.0 / np.sqrt(deg)).astype(np.float32)
    dinv_p = np.zeros(NP, np.float32)
    dinv_p[:N] = dinv

    xp = np.zeros((NP, F_IN), np.float16)
    xp[:N] = (dinv[:, None] * x).astype(np.float16)

    w1 = np.asarray(w1, np.float32)
    wcat = np.concatenate([w1[0], w1[1], w1[2]], axis=1).astype(np.float16)
    w2 = np.asarray(w2, np.float32)
    vcat = np.concatenate([w2[0], w2[1]], axis=1)  # [1536, 80]
    vt = np.ascontiguousarray(
        vcat.reshape(12, 128, 80).transpose(1, 0, 2).reshape(128, 12 * 80)
    ).astype(np.float16)
    b1w = np.ascontiguousarray(np.asarray(b1, np.float32).reshape(12, 128).T)
    b2 = np.asarray(b2, np.float32)
    b2a = b2[:40].reshape(40, 1).copy()
    b2b = b2[40:].reshape(40, 1).copy()

    iota512 = np.tile(np.arange(512, dtype=np.float16)[None, :], (128, 1))
    ident = np.eye(128, dtype=np.float16)
    ident32 = np.eye(128, dtype=np.float32)
    ones32 = np.ones((1, 128), np.float32)

    keep = row != col
    row, col = row[keep], col[keep]

    # ---- per-core edge cells, then cross-core-equalized chunk structure
    cores = []
    for c in range(N_CORES):
        lo = c * SL
        m = (col >= lo) & (col < lo + SL)
        r, d = row[m], col[m] - lo
        t4 = d >> 9
        w = r // WIN
        order = np.lexsort((r, w, t4))
        r, d, t4, w = r[order], d[order], t4[order], w[order]
        cell = t4 * NW + w
        counts = np.bincount(cell, minlength=NS4 * NW)
        starts = np.zeros(NS4 * NW + 1, np.int64)
        starts[1:] = np.cumsum(counts)
        cores.append(dict(r=r, d=d, counts=counts, starts=starts))

    all_counts = np.stack([cr["counts"] for cr in cores])          # [8, NS4*NW]
    cell_chunks = np.max(-(-all_counts // 128), axis=0)            # shared
    # chunk axis order: (s16, w, t4-within-s16, chunk)
    NS16 = -(-NS4 // 4)
    chunk_off = {}
    slab = {}      # (s16, w) -> (chunk_base, n_chunks)
    ctot = 0
    for s16 in range(NS16):
        t4s = range(s16 * 4, min(s16 * 4 + 4, NS4))
        for w in range(NW):
            base = ctot
            for t4 in t4s:
                chunk_off[(t4, w)] = ctot
                ctot += int(cell_chunks[t4 * NW + w])
            slab[(s16, w)] = (base, ctot - base)

    struct = dict(NS16=NS16, CTOT=ctot, cell_chunks=cell_chunks,
                  chunk_off=chunk_off, slab=slab)

    # ---- per-core idx / dlocal arrays in the shared layout
    per_core = []
    for c in range(N_CORES):
        cr = cores[c]
        idx_all = np.zeros((16, ctot * 8), np.int16)
        dl_all = np.full((128, ctot), -1.0, np.float16)
        for t4 in range(NS4):
            for w in range(NW):
                k = t4 * NW + w
                n = int(cr["counts"][k])
                if n == 0:
                    continue
                co = chunk_off[(t4, w)]
                npad = int(cell_chunks[k]) * 128
                a = cr["starts"][k]
                iseg = np.zeros(npad, np.int16)
                iseg[:n] = (cr["r"][a:a + n] - w * WIN).astype(np.int16)
                dseg = np.full(npad, -1.0, np.float16)
                dseg[:n] = (cr["d"][a:a + n] - t4 * 512).astype(np.float16)
                idx_all[:, co * 8:co * 8 + npad // 16] = _wrap16(iseg)
                dl_all[:, co:co + npad // 128] = dseg.reshape(-1, 128).T
        per_core.append(dict(idx=np.tile(idx_all, (8, 1)), dl=dl_all))

    # ---- per-core dense inputs
    for c in range(N_CORES):
        lo = c * SL
        hi = min(lo + SL, N)
        nr = hi - lo
        xT = np.zeros((128, SL), np.float16)
        xppT = np.zeros((128, SL), np.float16)
        if nr > 0:
            xT[:, :nr] = x[lo:hi].T.astype(np.float16)
            xppT[:, :nr] = (dinv[lo:hi][None, :] ** 3 * x[lo:hi].T).astype(np.float16)
        dv = dinv_p[lo:lo + SL]
        per_core[c].update(
            xT=xT, xppT=xppT,
            dinvrow=dv.reshape(1, SL).astype(np.float32),
            dinv2row=(dv * dv).reshape(1, SL).astype(np.float32),
            dinv4w=np.ascontiguousarray(dv.reshape(NT, 128).T.astype(np.float32)),
        )

    shared = dict(xp=xp, wcat=wcat, vt=vt, b1w=b1w, b2a=b2a, b2b=b2b,
                  iota512=iota512, ident=ident, ident32=ident32, ones32=ones32)
    return shared, per_core, struct


# ================================================================ builder

def build(dims, struct):
    SL, NP, NW, NS4, NT, NB = (dims[k] for k in ("SL", "NP", "NW", "NS4", "NT", "NB"))
    NS16, CTOT = struct["NS16"], struct["CTOT"]
    cell_chunks, chunk_off, slab = (struct[k] for k in ("cell_chunks", "chunk_off", "slab"))

    nc = bacc.Bacc("TRN2", target_bir_lowering=False, debug=False, num_devices=N_CORES)

    # DRAM tensors
    xp_d = nc.dram_tensor("xp", [NP, F_IN], FP16, kind="ExternalInput")
    idx_d = nc.dram_tensor("idx", [128, CTOT * 8], I16, kind="ExternalInput")
    dl_d = nc.dram_tensor("dl", [128, CTOT], FP16, kind="ExternalInput")
    xT_d = nc.dram_tensor("xT", [128, SL], FP16, kind="ExternalInput")
    xppT_d = nc.dram_tensor("xppT", [128, SL], FP16, kind="ExternalInput")
    dinvrow_d = nc.dram_tensor("dinvrow", [1, SL], F32, kind="ExternalInput")
    dinv2row_d = nc.dram_tensor("dinv2row", [1, SL], F32, kind="ExternalInput")
    dinv4w_d = nc.dram_tensor("dinv4w", [128, NT], F32, kind="ExternalInput")
    wcat_d = nc.dram_tensor("wcat", [128, 3 * HID], FP16, kind="ExternalInput")
    vt_d = nc.dram_tensor("vt", [128, 12 * 80], FP16, kind="ExternalInput")
    b1w_d = nc.dram_tensor("b1w", [128, 12], F32, kind="ExternalInput")
    b2a_d = nc.dram_tensor("b2a", [40, 1], F32, kind="ExternalInput")
    b2b_d = nc.dram_tensor("b2b", [40, 1], F32, kind="ExternalInput")
    iota_d = nc.dram_tensor("iota512", [128, 512], FP16, kind="ExternalInput")
    ident_d = nc.dram_tensor("ident", [128, 128], FP16, kind="ExternalInput")
    ident32_d = nc.dram_tensor("ident32", [128, 128], F32, kind="ExternalInput")
    ones32_d = nc.dram_tensor("ones32", [1, 128], F32, kind="ExternalInput")
    out_d = nc.dram_tensor("out", [SL, 80], F32, kind="ExternalOutput")

    cc_in1 = nc.dram_tensor("cc_in1", [SL, F_IN], FP16)
    cc_out1 = nc.dram_tensor("cc_out1", [NP, F_IN], FP16, addr_space="Shared")
    cc_in2 = nc.dram_tensor("cc_in2", [SL, 128], FP16)
    cc_out2 = nc.dram_tensor("cc_out2", [NP, 128], FP16, addr_space="Shared")
    z0T_d = nc.dram_tensor("z0T", [40, SL], F32)
    x1T_d = nc.dram_tensor("x1Tspill", [128, SL], FP16)
    x2T_d = nc.dram_tensor("x2Tspill", [128, SL], FP16)
    z1T_d = nc.dram_tensor("z1T", [40, SL], FP16)

    win_rows = [min(WIN, NP - w * WIN) for w in range(NW)]

    with tile.TileContext(nc) as tc, ExitStack() as ctx:
        cpool = ctx.enter_context(tc.tile_pool(name="consts", bufs=1))
        gpool = ctx.enter_context(tc.tile_pool(name="gslab", bufs=2))
        spool = ctx.enter_context(tc.tile_pool(name="sslab", bufs=2))
        ipool = ctx.enter_context(tc.tile_pool(name="idxs", bufs=2))
        appool = ctx.enter_context(tc.tile_pool(name="acts", bufs=1))
        epool = ctx.enter_context(tc.tile_pool(name="evac", bufs=2))
        hpool = ctx.enter_context(tc.tile_pool(name="hblk", bufs=2))
        ppool = ctx.enter_context(tc.tile_pool(name="psum", bufs=4, space="PSUM"))
        tpool = ctx.enter_context(tc.tile_pool(name="psum_t", bufs=2, space="PSUM"))

        # ---- constants / persistent slabs
        def load(shape, dt, src, nm):
            t = cpool.tile(shape, dt, tag=nm, name=nm)
            nc.sync.dma_start(out=t[:], in_=src[:])
            return t

        dl_sb = load([128, CTOT], FP16, dl_d, "c_dl")
        dinv4w = load([128, NT], F32, dinv4w_d, "c_dinv4w")
        wcat = load([128, 3 * HID], FP16, wcat_d, "c_wcat")
        vt = load([128, 12 * 80], FP16, vt_d, "c_vt")
        b1w = load([128, 12], F32, b1w_d, "c_b1w")
        b2a = load([40, 1], F32, b2a_d, "c_b2a")
        b2b = load([40, 1], F32, b2b_d, "c_b2b")
        iota = load([128, 512], FP16, iota_d, "c_iota")
        ident = load([128, 128], FP16, ident_d, "c_ident")
        ident32 = load([128, 128], F32, ident32_d, "c_ident32")
        ones32 = load([1, 128], F32, ones32_d, "c_ones32")

        def repl_row(row_dram, c0, w):
            rowwin = epool.tile([1, w], F32, tag="rowwin", name="rowwin")
            nc.sync.dma_start(out=rowwin[:], in_=row_dram[:, c0:c0 + w])
            r = tpool.tile([128, w], F32, tag="tp", name="replrow")
            nc.tensor.matmul(out=r[:], lhsT=ones32[:], rhs=rowwin[:], start=True, stop=True)
            return r
        x2T = appool.tile([128, SL], FP16, tag="x2T")

        def prop(src_dram, elem, lhs_feats, evac_s4):
            """One propagation: gathers + one-hot scatter matmuls, then per-s4
            evacuation. src rows are [elem] fp16 (256B). evac_s4(t4, acc) with
            acc = PSUM [lhs_feats, 512] f32 accumulated A-sum (feature-major)."""
            for s16 in range(NS16):
                t4s = list(range(s16 * 4, min(s16 * 4 + 4, NS4)))
                accs = {t4: ppool.tile([128, 512], F32, tag="acc", name=f"acc{t4}") for t4 in t4s}
                first = {t4: True for t4 in t4s}
                for w in range(NW):
                    base, nch = slab[(s16, w)]
                    if nch == 0:
                        continue
                    g = gpool.tile([128, nch, elem], FP16, tag="g")
                    idx_sb = ipool.tile([128, nch * 8], I16, tag="idx")
                    nc.sync.dma_start(out=idx_sb[:], in_=idx_d[:, base * 8:(base + nch) * 8])
                    for k0 in range(0, nch, 8):
                        kn = min(8, nch - k0)
                        nc.gpsimd.dma_gather(
                            out_ap=g[:, k0:k0 + kn, :],
                            in_ap=src_dram[w * WIN: w * WIN + win_rows[w], :],
                            idxs_ap=idx_sb[:, k0 * 8:(k0 + kn) * 8],
                            num_idxs=kn * 128,
                            num_idxs_reg=kn * 128,
                            elem_size=elem,
                        )
                    for t4 in t4s:
                        co = chunk_off[(t4, w)]
                        ncell = int(cell_chunks[t4 * NW + w])
                        if ncell == 0:
                            continue
                        s = spool.tile([128, ncell, 512], FP16, tag="s")
                        nc.vector.tensor_tensor(
                            out=s[:],
                            in0=dl_sb[:, co:co + ncell].unsqueeze(-1).broadcast_to((128, ncell, 512)),
                            in1=iota[:].unsqueeze(1).broadcast_to((128, ncell, 512)),
                            op=mybir.AluOpType.is_equal,
                        )
                        for ci in range(ncell):
                            gi = co - base + ci
                            nc.tensor.matmul(
                                out=accs[t4][:lhs_feats, :],
                                lhsT=g[:, gi, :lhs_feats],
                                rhs=s[:, ci, :],
                                start=first[t4],
                                stop=(w == NW - 1 or all(
                                    cell_chunks[t4 * NW + w2] == 0 for w2 in range(w + 1, NW)
                                )) and ci == ncell - 1,
                            )
                            first[t4] = False
                for t4 in t4s:
                    if all(cell_chunks[t4 * NW + w] == 0 for w in range(NW)):
                        nc.vector.memset(accs[t4][:], 0.0)
                    evac_s4(t4, accs[t4])

        # ================= P1: u1 = A x'   (feature-major accumulate)
        def evac_p1(t4, acc):
            c0 = t4 * 512
            xpp_blk = epool.tile([128, 512], FP16, tag="xpp")
            nc.sync.dma_start(out=xpp_blk[:], in_=xppT_d[:, c0:c0 + 512])
            # x1T = dinv_row * u1 + x''T      (x1 = D u1 + D^2 x'; feature-major)
            rd = repl_row(dinvrow_d, c0, 512)
            tmp = epool.tile([128, 512], F32, tag="ev32")
            nc.vector.tensor_copy(out=tmp[:], in_=acc[:])
            nc.vector.tensor_tensor(
                out=tmp[:], in0=tmp[:], in1=rd[:], op=mybir.AluOpType.mult)
            x1blk = epool.tile([128, 512], FP16, tag="x1blk")
            nc.vector.tensor_tensor(
                out=x1blk[:], in0=tmp[:], in1=xpp_blk[:],
                op=mybir.AluOpType.add)
            nc.sync.dma_start(out=x1T_d[:, c0:c0 + 512], in_=x1blk[:])
            # x1' = dinv * x1 (node-major) -> cc_in1
            pt = tpool.tile([128, 4, 128], FP16, tag="tp16")
            for t in range(4):
                nc.tensor.transpose(
                    out=pt[:, t, :], in_=x1blk[:, t * 128:(t + 1) * 128],
                    identity=ident[:])
            x1n = epool.tile([128, 4, 128], FP16, tag="x1n")
            nc.vector.tensor_tensor(
                out=x1n[:], in0=pt[:],
                in1=dinv4w[:, t4 * 4: t4 * 4 + 4].unsqueeze(-1).broadcast_to((128, 4, 128)),
                op=mybir.AluOpType.mult)
            nc.sync.dma_start(
                out=cc_in1[c0:c0 + 512, :].rearrange("(a p) b -> p a b", p=128),
                in_=x1n[:])

        prop(xp_d, 128, 128, evac_p1)

        nc.gpsimd.collective_compute(
            "AllGather", mybir.AluOpType.bypass,
            ins=[cc_in1[:]], outs=[cc_out1[:]],
            replica_groups=[list(range(N_CORES))])

        # ================= P2: u2 = A x1'
        def evac_p2(t4, acc):
            c0 = t4 * 512
            rd = repl_row(dinvrow_d, c0, 512)
            t1 = epool.tile([128, 512], F32, tag="ev32")
            nc.vector.tensor_copy(out=t1[:], in_=acc[:])
            nc.vector.tensor_tensor(
                out=t1[:], in0=t1[:], in1=rd[:], op=mybir.AluOpType.mult)
            rd2 = repl_row(dinv2row_d, c0, 512)
            x1rd = epool.tile([128, 512], FP16, tag="x1blk")
            nc.sync.dma_start(out=x1rd[:], in_=x1T_d[:, c0:c0 + 512])
            t2 = epool.tile([128, 512], F32, tag="ev32b")
            nc.vector.tensor_tensor(
                out=t2[:], in0=x1rd[:], in1=rd2[:],
                op=mybir.AluOpType.mult)
            x2blk = epool.tile([128, 512], FP16, tag="x2blk")
            nc.vector.tensor_tensor(
                out=x2blk[:], in0=t1[:], in1=t2[:],
                op=mybir.AluOpType.add)
            nc.sync.dma_start(out=x2T_d[:, c0:c0 + 512], in_=x2blk[:])

        prop(cc_out1, 128, 128, evac_p2)

        # ================= phase 4: GEMMs + z + z1-node prep
        for b in range(NB):
            c0 = b * NBLK
            xT_blk = epool.tile([128, NBLK], FP16, tag="xTblk")
            nc.sync.dma_start(out=xT_blk[:], in_=xT_d[:, c0:c0 + NBLK])
            x1T_blk = epool.tile([128, NBLK], FP16, tag="x1blk")
            nc.sync.dma_start(out=x1T_blk[:], in_=x1T_d[:, c0:c0 + NBLK])
            x2T_blk = epool.tile([128, NBLK], FP16, tag="x2blk")
            nc.sync.dma_start(out=x2T_blk[:], in_=x2T_d[:, c0:c0 + NBLK])
            h_sb = hpool.tile([128, 12, NBLK], FP16, tag="h")
            for kt in range(12):
                src = (xT_blk[:], x1T_blk[:], x2T_blk[:])[kt // 4]
                ph = tpool.tile([128, NBLK], F32, tag="tp")
                nc.tensor.matmul(
                    out=ph[:], lhsT=wcat[:, kt * 128:(kt + 1) * 128], rhs=src,
                    start=True, stop=True)
                nc.vector.tensor_scalar(
                    out=h_sb[:, kt, :], in0=ph[:],
                    scalar1=b1w[:, kt:kt + 1], scalar2=0.0,
                    op0=mybir.AluOpType.add, op1=mybir.AluOpType.max)
            pz0 = tpool.tile([128, NBLK], F32, tag="tp", name="pz0")
            pz1 = tpool.tile([128, NBLK], F32, tag="tp", name="pz1")
            for kt in range(12):
                nc.tensor.matmul(
                    out=pz0[:40, :], lhsT=vt[:, kt * 80:kt * 80 + 40],
                    rhs=h_sb[:, kt, :], start=(kt == 0), stop=(kt == 11))
                nc.tensor.matmul(
                    out=pz1[:40, :], lhsT=vt[:, kt * 80 + 40:(kt + 1) * 80],
                    rhs=h_sb[:, kt, :], start=(kt == 0), stop=(kt == 11))
            z0sb = epool.tile([40, NBLK], F32, tag="z0sb")
            nc.vector.tensor_copy(out=z0sb[:], in_=pz0[0:40, :])
            nc.sync.dma_start(out=z0T_d[:, c0:c0 + NBLK], in_=z0sb[:])
            z1Tt = epool.tile([40, NBLK], FP16, tag="z1Tt")
            nc.vector.tensor_copy(out=z1Tt[:], in_=pz1[0:40, :])
            nc.sync.dma_start(out=z1T_d[:, c0:c0 + NBLK], in_=z1Tt[:])
            # node-major z1' = dinv * z1, padded to 128 cols
            zt = tpool.tile([128, 4, 64], FP16, tag="tp16")
            for t in range(4):
                nc.tensor.transpose(
                    out=zt[:, t, 0:40], in_=z1Tt[:, t * 128:(t + 1) * 128],
                    identity=ident[:40, :40])
            z1n = epool.tile([128, 4, 128], FP16, tag="z1n")
            nc.vector.memset(z1n[:], 0.0)
            t4b = c0 // 128
            nc.vector.tensor_tensor(
                out=z1n[:, :, 0:40], in0=zt[:, :, 0:40],
                in1=dinv4w[:, t4b:t4b + 4].unsqueeze(-1).broadcast_to((128, 4, 40)),
                op=mybir.AluOpType.mult)
            nc.sync.dma_start(
                out=cc_in2[c0:c0 + 512, :].rearrange("(a p) b -> p a b", p=128),
                in_=z1n[:])

        nc.gpsimd.collective_compute(
            "AllGather", mybir.AluOpType.bypass,
            ins=[cc_in2[:]], outs=[cc_out2[:]],
            replica_groups=[list(range(N_CORES))])

        # ================= P3: u3 = A z1'  -> y -> log_softmax -> out
        def evac_p3(t4, acc):
            c0 = t4 * 512
            # y2T = dinv_row*u3 + dinv2_row*z1T + b2b   [40, 512] f32
            rd = repl_row(dinvrow_d, c0, 512)
            y2 = epool.tile([40, 512], F32, tag="y2")
            nc.vector.tensor_copy(out=y2[:], in_=acc[:40, :])
            nc.vector.tensor_tensor(
                out=y2[:], in0=y2[:], in1=rd[:40, :], op=mybir.AluOpType.mult)
            z1blk = epool.tile([40, 512], FP16, tag="z1blk")
            nc.sync.dma_start(out=z1blk[:], in_=z1T_d[:, c0:c0 + 512])
            rd2 = repl_row(dinv2row_d, c0, 512)
            t2 = epool.tile([40, 512], F32, tag="y2b")
            nc.vector.tensor_tensor(
                out=t2[:], in0=z1blk[:], in1=rd2[:40, :],
                op=mybir.AluOpType.mult)
            nc.vector.tensor_tensor(out=y2[:], in0=y2[:], in1=t2[:], op=mybir.AluOpType.add)
            nc.vector.tensor_scalar(
                out=y2[:], in0=y2[:], scalar1=b2b[:], scalar2=None,
                op0=mybir.AluOpType.add)
            # y1T = z0T + b2a
            y1 = epool.tile([40, 512], F32, tag="y1")
            nc.sync.dma_start(out=y1[:], in_=z0T_d[:, c0:c0 + 512])
            nc.vector.tensor_scalar(
                out=y1[:], in0=y1[:], scalar1=b2a[:], scalar2=None,
                op0=mybir.AluOpType.add)
            # node-major y [128, 4, 80]
            yt = tpool.tile([128, 4, 80], F32, tag="tp")
            for t in range(4):
                nc.tensor.transpose(
                    out=yt[:, t, 0:40], in_=y1[:, t * 128:(t + 1) * 128],
                    identity=ident32[:40, :40])
                nc.tensor.transpose(
                    out=yt[:, t, 40:80], in_=y2[:, t * 128:(t + 1) * 128],
                    identity=ident32[:40, :40])
            # log_softmax over last axis
            y_sb = epool.tile([128, 4, 80], F32, tag="ysb")
            mx = epool.tile([128, 4, 1], F32, tag="mx")
            nc.vector.tensor_reduce(
                out=mx[:], in_=yt[:], axis=mybir.AxisListType.X,
                op=mybir.AluOpType.max)
            nc.vector.tensor_tensor(
                out=y_sb[:], in0=yt[:], in1=mx[:].broadcast_to((128, 4, 80)),
                op=mybir.AluOpType.subtract)
            ex = epool.tile([128, 4, 80], F32, tag="ex")
            nc.scalar.activation(
                out=ex[:], in_=y_sb[:], func=mybir.ActivationFunctionType.Exp)
            sm = epool.tile([128, 4, 1], F32, tag="sm")
            nc.vector.tensor_reduce(
                out=sm[:], in_=ex[:], axis=mybir.AxisListType.X,
                op=mybir.AluOpType.add)
            ls = epool.tile([128, 4, 1], F32, tag="ls")
            nc.scalar.activation(
                out=ls[:], in_=sm[:], func=mybir.ActivationFunctionType.Ln)
            nc.vector.tensor_tensor(
                out=y_sb[:], in0=y_sb[:], in1=ls[:].broadcast_to((128, 4, 80)),
                op=mybir.AluOpType.subtract)
            nc.sync.dma_start(
                out=out_d[c0:c0 + 512, :].rearrange("(a p) b -> p a b", p=128),
                in_=y_sb[:])

        prop(cc_out2, 128, 40, evac_p3)

    nc.compile()
    return nc


# ================================================================ entry

def kernel(x, edge_index, w1, b1, w2, b2):
    n = x.shape[0]
    dims = make_dims(n)
    shared, per_core, struct = host_prep(x, edge_index, w1, b1, w2, b2, dims)
    nc = build(dims, struct)
    in_maps = []
    for c in range(N_CORES):
        pc = per_core[c]
        in_maps.append(dict(
            xp=shared["xp"], idx=pc["idx"], dl=pc["dl"],
            xT=pc["xT"], xppT=pc["xppT"],
            dinvrow=pc["dinvrow"], dinv2row=pc["dinv2row"], dinv4w=pc["dinv4w"],
            wcat=shared["wcat"], vt=shared["vt"], b1w=shared["b1w"],
            b2a=shared["b2a"], b2b=shared["b2b"], iota512=shared["iota512"], ident=shared["ident"],
            ident32=shared["ident32"], ones32=shared["ones32"],
        ))
    res = bass_utils.run_bass_kernel_spmd(nc, in_maps, core_ids=list(range(N_CORES)))
    out = np.concatenate([res.results[c]["out"] for c in range(N_CORES)], axis=0)
    return np.ascontiguousarray(out[:n]).astype(np.float32)



# revision 8
# speedup vs baseline: 1.6017x; 1.6017x over previous
"""MixHopNet Trainium2 kernel: 8-core SPMD, node-sharded.

Math (matches the jax reference):
  dinv = rsqrt(deg) with self loops; Ahat = D (A+I) D  (D = diag(dinv))
  h  = relu([x W0, (Ahat x) W1, (Ahat^2 x) W2] + b1)
  y  = [h V0, Ahat (h V1)] + b2
  out = log_softmax(y, axis=1)

Implementation notes:
  - norm is separable (dinv[row]*dinv[col]) -> propagation is a 0/1 adjacency
    sum over pre-scaled features; self loops are fused scale-adds at PSUM evac.
  - gather: gpsimd.dma_gather (int16 idxs -> 32k-row windows), fp16 rows 256B.
    Descriptor generation is the bottleneck engine, so gathers are batched
    (16 chunks = 2048 idxs per call, bounded by the 256-desc/engine ring) and
    spread round-robin over all 4 SWDGE queues (each queue = its own Q7 core
    pair) for 4x parallel descriptor generation.
  - scatter: one-hot matmul on TensorE. S[128 edges, 256 dests] generated on
    VectorE via is_equal(dlocal, iota256); PSUM out is feature-major
    [128 feat, 256 dest] accumulated over all chunks of a 256-dest cell.
  - one program for all 8 cores: per-(cell, window) chunk counts are
    equalized to the cross-core max (pad slots gather row 0 and have
    dlocal=-1 so they contribute nothing).
  - all-gathers of the propagated features between steps (ncfw collectives).
"""

from contextlib import ExitStack

import numpy as np
import ml_dtypes

import concourse.bass as bass
import concourse.mybir as mybir
import concourse.tile as tile
from concourse import bacc, bass_utils

FP16 = mybir.dt.float16
F32 = mybir.dt.float32
I16 = mybir.dt.int16

N_CORES = 8
F_IN, HID, NCLS = 128, 512, 40
WIN = 32768
NBLK = 512
DW = 256                 # dest-cell width (one-hot width, matmul N)
GRP = 8                  # cells per slab group (8*256 = 2048 dests)
import os
# chunks per dma_gather call: 8 chunks = 1024 idxs = 65 descs/engine/side,
# sized to the SWDGE descriptor ring (16-chunk calls overflow it and hang).
CALL_CHUNKS = int(os.environ.get("MH_CALL_CHUNKS", "8"))
NQ = int(os.environ.get("MH_NQ", "4"))                     # SWDGE queues


def make_dims(n):
    sl = -(-n // (N_CORES * 512)) * 512       # per-core slice, multiple of 512
    npad = sl * N_CORES
    return dict(
        N=n, SL=sl, NP=npad,
        NT=sl // 128,            # 128-row dest tiles per core
        NSC=sl // DW,            # 256-dest cells per core
        NW=-(-npad // WIN),      # source windows
        NB=sl // NBLK,           # GEMM n-blocks
    )


# ================================================================ host prep

def _wrap16(seg):
    return np.ascontiguousarray(seg.reshape(-1, 16).T)


def host_prep(x, edge_index, w1, b1, w2, b2, dims):
    N, SL, NP, NSC, NW = dims["N"], dims["SL"], dims["NP"], dims["NSC"], dims["NW"]
    NT = dims["NT"]
    x = np.asarray(x, np.float32)
    ei = np.asarray(edge_index)
    row, col = ei[0].astype(np.int64), ei[1].astype(np.int64)

    deg = np.bincount(col, minlength=N).astype(np.float32) + 1.0
    dinv = (1.0 / np.sqrt(deg)).astype(np.float32)
    dinv_p = np.zeros(NP, np.float32)
    dinv_p[:N] = dinv

    xp = np.zeros((NP, F_IN), np.float16)
    xp[:N] = (dinv[:, None] * x).astype(np.float16)

    w1 = np.asarray(w1, np.float32)
    wcat = np.concatenate([w1[0], w1[1], w1[2]], axis=1).astype(np.float16)
    w2 = np.asarray(w2, np.float32)
    vcat = np.concatenate([w2[0], w2[1]], axis=1)  # [1536, 80]
    vt = np.ascontiguousarray(
        vcat.reshape(12, 128, 80).transpose(1, 0, 2).reshape(128, 12 * 80)
    ).astype(np.float16)
    b1w = np.ascontiguousarray(np.asarray(b1, np.float32).reshape(12, 128).T)
    b2 = np.asarray(b2, np.float32)
    b2a = b2[:40].reshape(40, 1).copy()
    b2b = b2[40:].reshape(40, 1).copy()

    iota = np.tile(np.arange(DW, dtype=np.float16)[None, :], (128, 1))
    ident = np.eye(128, dtype=np.float16)
    ident32 = np.eye(128, dtype=np.float32)
    ones32 = np.ones((1, 128), np.float32)

    keep = row != col
    row, col = row[keep], col[keep]

    # ---- per-core edge cells, then cross-core-equalized chunk structure
    cores = []
    for c in range(N_CORES):
        lo = c * SL
        m = (col >= lo) & (col < lo + SL)
        r, d = row[m], col[m] - lo
        cell = d // DW
        w = r // WIN
        order = np.lexsort((r, w, cell))
        r, d, cell, w = r[order], d[order], cell[order], w[order]
        key = cell * NW + w
        counts = np.bincount(key, minlength=NSC * NW)
        starts = np.zeros(NSC * NW + 1, np.int64)
        starts[1:] = np.cumsum(counts)
        cores.append(dict(r=r, d=d, counts=counts, starts=starts))

    all_counts = np.stack([cr["counts"] for cr in cores])          # [8, NSC*NW]
    cell_chunks = np.max(-(-all_counts // 128), axis=0)            # shared
    # chunk axis order: (sg, w, cell-within-sg, chunk)
    NSG = -(-NSC // GRP)
    chunk_off = {}
    slab = {}      # (sg, w) -> (chunk_base, n_chunks)
    ctot = 0
    for sg in range(NSG):
        cls = range(sg * GRP, min(sg * GRP + GRP, NSC))
        for w in range(NW):
            base = ctot
            for cl in cls:
                chunk_off[(cl, w)] = ctot
                ctot += int(cell_chunks[cl * NW + w])
            slab[(sg, w)] = (base, ctot - base)

    struct = dict(NSG=NSG, CTOT=ctot, cell_chunks=cell_chunks,
                  chunk_off=chunk_off, slab=slab)

    # ---- per-core idx / dlocal arrays in the shared layout
    per_core = []
    for c in range(N_CORES):
        cr = cores[c]
        idx_all = np.zeros((16, ctot * 8), np.int16)
        dl_all = np.full((128, ctot), -1.0, np.float16)
        for cl in range(NSC):
            for w in range(NW):
                k = cl * NW + w
                n = int(cr["counts"][k])
                if n == 0:
                    continue
                co = chunk_off[(cl, w)]
                npad = int(cell_chunks[k]) * 128
                a = cr["starts"][k]
                iseg = np.zeros(npad, np.int16)
                iseg[:n] = (cr["r"][a:a + n] - w * WIN).astype(np.int16)
                dseg = np.full(npad, -1.0, np.float16)
                dseg[:n] = (cr["d"][a:a + n] - cl * DW).astype(np.float16)
                idx_all[:, co * 8:co * 8 + npad // 16] = _wrap16(iseg)
                dl_all[:, co:co + npad // 128] = dseg.reshape(-1, 128).T
        per_core.append(dict(idx=np.tile(idx_all, (8, 1)), dl=dl_all))

    # ---- per-core dense inputs
    for c in range(N_CORES):
        lo = c * SL
        hi = min(lo + SL, N)
        nr = hi - lo
        xT = np.zeros((128, SL), np.float16)
        xppT = np.zeros((128, SL), np.float16)
        if nr > 0:
            xT[:, :nr] = x[lo:hi].T.astype(np.float16)
            xppT[:, :nr] = (dinv[lo:hi][None, :] ** 3 * x[lo:hi].T).astype(np.float16)
        dv = dinv_p[lo:lo + SL]
        per_core[c].update(
            xT=xT, xppT=xppT,
            dinvrow=dv.reshape(1, SL).astype(np.float32),
            dinv2row=(dv * dv).reshape(1, SL).astype(np.float32),
            dinv4w=np.ascontiguousarray(dv.reshape(NT, 128).T.astype(np.float32)),
        )

    shared = dict(xp=xp, wcat=wcat, vt=vt, b1w=b1w, b2a=b2a, b2b=b2b,
                  iota=iota, ident=ident, ident32=ident32, ones32=ones32)
    return shared, per_core, struct


# ================================================================ builder

def build(dims, struct):
    SL, NP, NW, NSC, NT, NB = (dims[k] for k in ("SL", "NP", "NW", "NSC", "NT", "NB"))
    NSG, CTOT = struct["NSG"], struct["CTOT"]
    cell_chunks, chunk_off, slab = (struct[k] for k in ("cell_chunks", "chunk_off", "slab"))

    nc = bacc.Bacc("TRN2", target_bir_lowering=False, debug=False,
                   num_devices=N_CORES, num_swdge_queues=NQ)

    # DRAM tensors
    xp_d = nc.dram_tensor("xp", [NP, F_IN], FP16, kind="ExternalInput")
    idx_d = nc.dram_tensor("idx", [128, CTOT * 8], I16, kind="ExternalInput")
    dl_d = nc.dram_tensor("dl", [128, CTOT], FP16, kind="ExternalInput")
    xT_d = nc.dram_tensor("xT", [128, SL], FP16, kind="ExternalInput")
    xppT_d = nc.dram_tensor("xppT", [128, SL], FP16, kind="ExternalInput")
    dinvrow_d = nc.dram_tensor("dinvrow", [1, SL], F32, kind="ExternalInput")
    dinv2row_d = nc.dram_tensor("dinv2row", [1, SL], F32, kind="ExternalInput")
    dinv4w_d = nc.dram_tensor("dinv4w", [128, NT], F32, kind="ExternalInput")
    wcat_d = nc.dram_tensor("wcat", [128, 3 * HID], FP16, kind="ExternalInput")
    vt_d = nc.dram_tensor("vt", [128, 12 * 80], FP16, kind="ExternalInput")
    b1w_d = nc.dram_tensor("b1w", [128, 12], F32, kind="ExternalInput")
    b2a_d = nc.dram_tensor("b2a", [40, 1], F32, kind="ExternalInput")
    b2b_d = nc.dram_tensor("b2b", [40, 1], F32, kind="ExternalInput")
    iota_d = nc.dram_tensor("iota", [128, DW], FP16, kind="ExternalInput")
    ident_d = nc.dram_tensor("ident", [128, 128], FP16, kind="ExternalInput")
    ident32_d = nc.dram_tensor("ident32", [128, 128], F32, kind="ExternalInput")
    ones32_d = nc.dram_tensor("ones32", [1, 128], F32, kind="ExternalInput")
    out_d = nc.dram_tensor("out", [SL, 80], F32, kind="ExternalOutput")

    cc_in1 = nc.dram_tensor("cc_in1", [SL, F_IN], FP16)
    cc_out1 = nc.dram_tensor("cc_out1", [NP, F_IN], FP16, addr_space="Shared")
    cc_in2 = nc.dram_tensor("cc_in2", [SL, 128], FP16)
    cc_out2 = nc.dram_tensor("cc_out2", [NP, 128], FP16, addr_space="Shared")
    z0T_d = nc.dram_tensor("z0T", [40, SL], F32)
    x1T_d = nc.dram_tensor("x1Tspill", [128, SL], FP16)
    x2T_d = nc.dram_tensor("x2Tspill", [128, SL], FP16)
    z1T_d = nc.dram_tensor("z1T", [40, SL], FP16)

    win_rows = [min(WIN, NP - w * WIN) for w in range(NW)]
    qctr = [0]                 # round-robin SWDGE queue assignment

    with tile.TileContext(nc) as tc, ExitStack() as ctx:
        cpool = ctx.enter_context(tc.tile_pool(name="consts", bufs=1))
        gpool = ctx.enter_context(tc.tile_pool(name="gslab", bufs=4))
        spool = ctx.enter_context(tc.tile_pool(name="sslab", bufs=3))
        ipool = ctx.enter_context(tc.tile_pool(name="idxs", bufs=4))
        epool = ctx.enter_context(tc.tile_pool(name="evac", bufs=2))
        hpool = ctx.enter_context(tc.tile_pool(name="hblk", bufs=2))
        ppool = ctx.enter_context(tc.tile_pool(name="psum", bufs=4, space="PSUM"))
        tpool = ctx.enter_context(tc.tile_pool(name="psum_t", bufs=2, space="PSUM"))

        # ---- constants / persistent slabs
        def load(shape, dt, src, nm):
            t = cpool.tile(shape, dt, tag=nm, name=nm)
            nc.sync.dma_start(out=t[:], in_=src[:])
            return t

        dl_sb = load([128, CTOT], FP16, dl_d, "c_dl")
        dinv4w = load([128, NT], F32, dinv4w_d, "c_dinv4w")
        wcat = load([128, 3 * HID], FP16, wcat_d, "c_wcat")
        vt = load([128, 12 * 80], FP16, vt_d, "c_vt")
        b1w = load([128, 12], F32, b1w_d, "c_b1w")
        b2a = load([40, 1], F32, b2a_d, "c_b2a")
        b2b = load([40, 1], F32, b2b_d, "c_b2b")
        iota = load([128, DW], FP16, iota_d, "c_iota")
        ident = load([128, 128], FP16, ident_d, "c_ident")
        ident32 = load([128, 128], F32, ident32_d, "c_ident32")
        ones32 = load([1, 128], F32, ones32_d, "c_ones32")

        def repl_row(row_dram, c0, w):
            rowwin = epool.tile([1, w], F32, tag="rowwin", name="rowwin")
            nc.sync.dma_start(out=rowwin[:], in_=row_dram[:, c0:c0 + w])
            r = tpool.tile([128, w], F32, tag="tp", name="replrow")
            nc.tensor.matmul(out=r[:], lhsT=ones32[:], rhs=rowwin[:], start=True, stop=True)
            return r

        def prop(src_dram, elem, lhs_feats, evac_cell):
            """One propagation: gathers + one-hot scatter matmuls, then per-cell
            evacuation. src rows are [elem] fp16 (256B). evac_cell(cl, acc) with
            acc = PSUM [lhs_feats, DW] f32 accumulated A-sum (feature-major)."""
            for sg in range(NSG):
                cls = list(range(sg * GRP, min(sg * GRP + GRP, NSC)))
                # Two cells share one PSUM bank ([128, 2, DW] f32 = 2 KiB).
                # A matmul with start=True clears has_written for the WHOLE
                # bank, so the pair forms ONE accumulation group: only the
                # pair's first matmul sets start, only its last sets stop
                # (flags=0 overwrites-where-unwritten, which acts as the
                # second cell's start).
                pair_of = {cl: j // 2 for j, cl in enumerate(cls)}
                npairs = (len(cls) + 1) // 2
                pairs = {p: ppool.tile([128, 2, DW], F32, tag="acc",
                                       name=f"accp{sg}_{p}") for p in range(npairs)}
                accs = {cl: pairs[pair_of[cl]][:, j % 2, :] for j, cl in enumerate(cls)}
                remaining = {p: 0 for p in range(npairs)}
                for j, cl in enumerate(cls):
                    remaining[pair_of[cl]] += int(
                        sum(cell_chunks[cl * NW + w] for w in range(NW)))
                first = {p: True for p in range(npairs)}
                for w in range(NW):
                    base, nch = slab[(sg, w)]
                    if nch == 0:
                        continue
                    g = gpool.tile([128, nch, elem], FP16, tag="g")
                    idx_sb = ipool.tile([128, nch * 8], I16, tag="idx")
                    nc.sync.dma_start(out=idx_sb[:], in_=idx_d[:, base * 8:(base + nch) * 8])
                    for k0 in range(0, nch, CALL_CHUNKS):
                        kn = min(CALL_CHUNKS, nch - k0)
                        nc.gpsimd.dma_gather(
                            out_ap=g[:, k0:k0 + kn, :],
                            in_ap=src_dram[w * WIN: w * WIN + win_rows[w], :],
                            idxs_ap=idx_sb[:, k0 * 8:(k0 + kn) * 8],
                            num_idxs=kn * 128,
                            num_idxs_reg=kn * 128,
                            elem_size=elem,
                            queue_num=qctr[0] % NQ,
                        )
                        qctr[0] += 1
                    for cl in cls:
                        co = chunk_off[(cl, w)]
                        ncell = int(cell_chunks[cl * NW + w])
                        if ncell == 0:
                            continue
                        p = pair_of[cl]
                        s = spool.tile([128, ncell, DW], FP16, tag="s")
                        nc.vector.tensor_tensor(
                            out=s[:],
                            in0=dl_sb[:, co:co + ncell].unsqueeze(-1).broadcast_to((128, ncell, DW)),
                            in1=iota[:].unsqueeze(1).broadcast_to((128, ncell, DW)),
                            op=mybir.AluOpType.is_equal,
                        )
                        for ci in range(ncell):
                            gi = co - base + ci
                            remaining[p] -= 1
                            nc.tensor.matmul(
                                out=accs[cl][:lhs_feats, :],
                                lhsT=g[:, gi, :lhs_feats],
                                rhs=s[:, ci, :],
                                start=first[p],
                                stop=remaining[p] == 0,
                            )
                            first[p] = False
                for cl in cls:
                    if all(cell_chunks[cl * NW + w] == 0 for w in range(NW)):
                        nc.vector.memset(accs[cl][:], 0.0)
                    evac_cell(cl, accs[cl])

        # ================= P1: u1 = A x'   (feature-major accumulate)
        def evac_p1(cl, acc):
            c0 = cl * DW
            xpp_blk = epool.tile([128, DW], FP16, tag="xpp")
            nc.sync.dma_start(out=xpp_blk[:], in_=xppT_d[:, c0:c0 + DW])
            # x1T = dinv_row * u1 + x''T      (x1 = D u1 + D^2 x'; feature-major)
            rd = repl_row(dinvrow_d, c0, DW)
            tmp = epool.tile([128, DW], F32, tag="ev32")
            nc.vector.tensor_copy(out=tmp[:], in_=acc[:])
            nc.vector.tensor_tensor(
                out=tmp[:], in0=tmp[:], in1=rd[:], op=mybir.AluOpType.mult)
            x1blk = epool.tile([128, DW], FP16, tag="x1blk")
            nc.vector.tensor_tensor(
                out=x1blk[:], in0=tmp[:], in1=xpp_blk[:],
                op=mybir.AluOpType.add)
            nc.sync.dma_start(out=x1T_d[:, c0:c0 + DW], in_=x1blk[:])
            # x1' = dinv * x1 (node-major) -> cc_in1
            nt = DW // 128
            pt = tpool.tile([128, nt, 128], FP16, tag="tp16")
            for t in range(nt):
                nc.tensor.transpose(
                    out=pt[:, t, :], in_=x1blk[:, t * 128:(t + 1) * 128],
                    identity=ident[:])
            x1n = epool.tile([128, nt, 128], FP16, tag="x1n")
            t0 = cl * nt
            nc.vector.tensor_tensor(
                out=x1n[:], in0=pt[:],
                in1=dinv4w[:, t0:t0 + nt].unsqueeze(-1).broadcast_to((128, nt, 128)),
                op=mybir.AluOpType.mult)
            nc.sync.dma_start(
                out=cc_in1[c0:c0 + DW, :].rearrange("(a p) b -> p a b", p=128),
                in_=x1n[:])

        prop(xp_d, 128, 128, evac_p1)

        nc.gpsimd.collective_compute(
            "AllGather", mybir.AluOpType.bypass,
            ins=[cc_in1[:]], outs=[cc_out1[:]],
            replica_groups=[list(range(N_CORES))])

        # ================= P2: u2 = A x1'
        def evac_p2(cl, acc):
            c0 = cl * DW
            rd = repl_row(dinvrow_d, c0, DW)
            t1 = epool.tile([128, DW], F32, tag="ev32")
            nc.vector.tensor_copy(out=t1[:], in_=acc[:])
            nc.vector.tensor_tensor(
                out=t1[:], in0=t1[:], in1=rd[:], op=mybir.AluOpType.mult)
            rd2 = repl_row(dinv2row_d, c0, DW)
            x1rd = epool.tile([128, DW], FP16, tag="x1blk")
            nc.sync.dma_start(out=x1rd[:], in_=x1T_d[:, c0:c0 + DW])
            t2 = epool.tile([128, DW], F32, tag="ev32b")
            nc.vector.tensor_tensor(
                out=t2[:], in0=x1rd[:], in1=rd2[:],
                op=mybir.AluOpType.mult)
            x2blk = epool.tile([128, DW], FP16, tag="x2blk")
            nc.vector.tensor_tensor(
                out=x2blk[:], in0=t1[:], in1=t2[:],
                op=mybir.AluOpType.add)
            nc.sync.dma_start(out=x2T_d[:, c0:c0 + DW], in_=x2blk[:])

        prop(cc_out1, 128, 128, evac_p2)

        # ================= phase 4: GEMMs + z + z1-node prep
        for b in range(NB):
            c0 = b * NBLK
            xT_blk = epool.tile([128, NBLK], FP16, tag="xTblk")
            nc.sync.dma_start(out=xT_blk[:], in_=xT_d[:, c0:c0 + NBLK])
            x1T_blk = epool.tile([128, NBLK], FP16, tag="x1gblk")
            nc.sync.dma_start(out=x1T_blk[:], in_=x1T_d[:, c0:c0 + NBLK])
            x2T_blk = epool.tile([128, NBLK], FP16, tag="x2gblk")
            nc.sync.dma_start(out=x2T_blk[:], in_=x2T_d[:, c0:c0 + NBLK])
            h_sb = hpool.tile([128, 12, NBLK], FP16, tag="h")
            for kt in range(12):
                src = (xT_blk[:], x1T_blk[:], x2T_blk[:])[kt // 4]
                ph = tpool.tile([128, NBLK], F32, tag="tp")
                nc.tensor.matmul(
                    out=ph[:], lhsT=wcat[:, kt * 128:(kt + 1) * 128], rhs=src,
                    start=True, stop=True)
                nc.vector.tensor_scalar(
                    out=h_sb[:, kt, :], in0=ph[:],
                    scalar1=b1w[:, kt:kt + 1], scalar2=0.0,
                    op0=mybir.AluOpType.add, op1=mybir.AluOpType.max)
            pz0 = tpool.tile([128, NBLK], F32, tag="tp", name="pz0")
            pz1 = tpool.tile([128, NBLK], F32, tag="tp", name="pz1")
            for kt in range(12):
                nc.tensor.matmul(
                    out=pz0[:40, :], lhsT=vt[:, kt * 80:kt * 80 + 40],
                    rhs=h_sb[:, kt, :], start=(kt == 0), stop=(kt == 11))
                nc.tensor.matmul(
                    out=pz1[:40, :], lhsT=vt[:, kt * 80 + 40:(kt + 1) * 80],
                    rhs=h_sb[:, kt, :], start=(kt == 0), stop=(kt == 11))
            z0sb = epool.tile([40, NBLK], F32, tag="z0sb")
            nc.vector.tensor_copy(out=z0sb[:], in_=pz0[0:40, :])
            nc.sync.dma_start(out=z0T_d[:, c0:c0 + NBLK], in_=z0sb[:])
            z1Tt = epool.tile([40, NBLK], FP16, tag="z1Tt")
            nc.vector.tensor_copy(out=z1Tt[:], in_=pz1[0:40, :])
            nc.sync.dma_start(out=z1T_d[:, c0:c0 + NBLK], in_=z1Tt[:])
            # node-major z1' = dinv * z1, padded to 128 cols
            zt = tpool.tile([128, 4, 64], FP16, tag="tp16")
            for t in range(4):
                nc.tensor.transpose(
                    out=zt[:, t, 0:40], in_=z1Tt[:, t * 128:(t + 1) * 128],
                    identity=ident[:40, :40])
            z1n = epool.tile([128, 4, 128], FP16, tag="z1n")
            nc.vector.memset(z1n[:], 0.0)
            t4b = c0 // 128
            nc.vector.tensor_tensor(
                out=z1n[:, :, 0:40], in0=zt[:, :, 0:40],
                in1=dinv4w[:, t4b:t4b + 4].unsqueeze(-1).broadcast_to((128, 4, 40)),
                op=mybir.AluOpType.mult)
            nc.sync.dma_start(
                out=cc_in2[c0:c0 + 512, :].rearrange("(a p) b -> p a b", p=128),
                in_=z1n[:])

        nc.gpsimd.collective_compute(
            "AllGather", mybir.AluOpType.bypass,
            ins=[cc_in2[:]], outs=[cc_out2[:]],
            replica_groups=[list(range(N_CORES))])

        # ================= P3: u3 = A z1'  -> y -> log_softmax -> out
        def evac_p3(cl, acc):
            c0 = cl * DW
            # y2T = dinv_row*u3 + dinv2_row*z1T + b2b   [40, DW] f32
            rd = repl_row(dinvrow_d, c0, DW)
            y2 = epool.tile([40, DW], F32, tag="y2")
            nc.vector.tensor_copy(out=y2[:], in_=acc[:40, :])
            nc.vector.tensor_tensor(
                out=y2[:], in0=y2[:], in1=rd[:40, :], op=mybir.AluOpType.mult)
            z1blk = epool.tile([40, DW], FP16, tag="z1blk")
            nc.sync.dma_start(out=z1blk[:], in_=z1T_d[:, c0:c0 + DW])
            rd2 = repl_row(dinv2row_d, c0, DW)
            t2 = epool.tile([40, DW], F32, tag="y2b")
            nc.vector.tensor_tensor(
                out=t2[:], in0=z1blk[:], in1=rd2[:40, :],
                op=mybir.AluOpType.mult)
            nc.vector.tensor_tensor(out=y2[:], in0=y2[:], in1=t2[:], op=mybir.AluOpType.add)
            nc.vector.tensor_scalar(
                out=y2[:], in0=y2[:], scalar1=b2b[:], scalar2=None,
                op0=mybir.AluOpType.add)
            # y1T = z0T + b2a
            y1 = epool.tile([40, DW], F32, tag="y1")
            nc.sync.dma_start(out=y1[:], in_=z0T_d[:, c0:c0 + DW])
            nc.vector.tensor_scalar(
                out=y1[:], in0=y1[:], scalar1=b2a[:], scalar2=None,
                op0=mybir.AluOpType.add)
            # node-major y [128, nt, 80]
            nt = DW // 128
            yt = tpool.tile([128, nt, 80], F32, tag="tp")
            for t in range(nt):
                nc.tensor.transpose(
                    out=yt[:, t, 0:40], in_=y1[:, t * 128:(t + 1) * 128],
                    identity=ident32[:40, :40])
                nc.tensor.transpose(
                    out=yt[:, t, 40:80], in_=y2[:, t * 128:(t + 1) * 128],
                    identity=ident32[:40, :40])
            # log_softmax over last axis
            y_sb = epool.tile([128, nt, 80], F32, tag="ysb")
            mx = epool.tile([128, nt, 1], F32, tag="mx")
            nc.vector.tensor_reduce(
                out=mx[:], in_=yt[:], axis=mybir.AxisListType.X,
                op=mybir.AluOpType.max)
            nc.vector.tensor_tensor(
                out=y_sb[:], in0=yt[:], in1=mx[:].broadcast_to((128, nt, 80)),
                op=mybir.AluOpType.subtract)
            ex = epool.tile([128, nt, 80], F32, tag="ex")
            nc.scalar.activation(
                out=ex[:], in_=y_sb[:], func=mybir.ActivationFunctionType.Exp)
            sm = epool.tile([128, nt, 1], F32, tag="sm")
            nc.vector.tensor_reduce(
                out=sm[:], in_=ex[:], axis=mybir.AxisListType.X,
                op=mybir.AluOpType.add)
            ls = epool.tile([128, nt, 1], F32, tag="ls")
            nc.scalar.activation(
                out=ls[:], in_=sm[:], func=mybir.ActivationFunctionType.Ln)
            nc.vector.tensor_tensor(
                out=y_sb[:], in0=y_sb[:], in1=ls[:].broadcast_to((128, nt, 80)),
                op=mybir.AluOpType.subtract)
            nc.sync.dma_start(
                out=out_d[c0:c0 + DW, :].rearrange("(a p) b -> p a b", p=128),
                in_=y_sb[:])

        prop(cc_out2, 128, 40, evac_p3)

    nc.compile()
    return nc


# ================================================================ entry

def kernel(x, edge_index, w1, b1, w2, b2):
    n = x.shape[0]
    dims = make_dims(n)
    shared, per_core, struct = host_prep(x, edge_index, w1, b1, w2, b2, dims)
    nc = build(dims, struct)
    in_maps = []
    for c in range(N_CORES):
        pc = per_core[c]
        in_maps.append(dict(
            xp=shared["xp"], idx=pc["idx"], dl=pc["dl"],
            xT=pc["xT"], xppT=pc["xppT"],
            dinvrow=pc["dinvrow"], dinv2row=pc["dinv2row"], dinv4w=pc["dinv4w"],
            wcat=shared["wcat"], vt=shared["vt"], b1w=shared["b1w"],
            b2a=shared["b2a"], b2b=shared["b2b"], iota=shared["iota"], ident=shared["ident"],
            ident32=shared["ident32"], ones32=shared["ones32"],
        ))
    res = bass_utils.run_bass_kernel_spmd(nc, in_maps, core_ids=list(range(N_CORES)))
    out = np.concatenate([res.results[c]["out"] for c in range(N_CORES)], axis=0)
    return np.ascontiguousarray(out[:n]).astype(np.float32)
